# revision 14
# baseline (speedup 1.0000x reference)
"""nn_KNN Trainium2 kernel: sequential per-node neighbor-mean scan as one GEMM.

The reference's scan is a composition of per-column linear updates, so
out = x0 @ M for a precomputable M. Folding the initial mask-fill into M
(zeroing the unknown rows -> M', bias r), known columns pass through
exactly and only the 256 unknown columns need compute:

  out[:, known]   = input[:, known]          (host-side pass-through)
  out[:, unknown] = input[:, known] @ Vk + r,  Vk = M'[known][:, unknown]

Sharding: batch b -> core b (data parallel, no collectives). Each core
gets its shard's known rows pre-tiled in fp8e4 as 8 time-slices
xP [8, 128, 6*512] (partition-contiguous 3KB DMA lines), plus Vk fp8e4.
It computes outU [256, 4096] = Vk.T @ xT + r with DoubleRow fp8 matmuls
(2 contraction rows per pass, double-pumped moving data) and writes outU
in bf16. Host: out = input.copy(); out[:, :, unknown] = outU.T.

Pipelining: the 8 slice loads all issue up-front, alternating between
the two HWDGE queues (sync=qSP, scalar=qAct); the first matmul starts
after one slice. Output stores go exclusively through gpsimd's SWDGE
queue so they never sit behind input loads. Bias-add + f32->bf16 cast
alternates vector/scalar.
"""

import sys

import numpy as np

try:
    import concourse.bass  # noqa: F401
except ImportError:  # pragma: no cover
    sys.path.insert(0, "/opt/trn_rl_repo")

import ml_dtypes

import concourse.bacc as bacc_mod
import concourse.mybir as mybir
from concourse.bass_utils import run_bass_kernel_spmd
from concourse.tile import TileContext

B, T, N, NS = 8, 4096, 1024, 256
NK = N - NS
P = 128
TW = 512
NL = T // TW
JC = NK // P        # 6 contraction chunks of 128

FP8 = ml_dtypes.float8_e4m3


def _build_kernel(ps_bufs=4, ot_bufs=4):
    nc = bacc_mod.Bacc("TRN2", target_bir_lowering=False, name="knn_fp8")
    f32 = mybir.dt.float32
    bf16 = mybir.dt.bfloat16
    fp8 = mybir.dt.float8e4
    xP = nc.dram_tensor("xP", [NL, P, JC * TW], fp8, kind="ExternalInput")
    Vk = nc.dram_tensor("Vk", [NK, NS], fp8, kind="ExternalInput")
    rb = nc.dram_tensor("r", [NS], f32, kind="ExternalInput")
    outU = nc.dram_tensor("outU", [NS, T], bf16, kind="ExternalOutput")

    CP = JC // 2        # 3 DoubleRow chunk-pairs
    SB = NS // P        # 2 output partition blocks

    with TileContext(nc) as tc:
        with (
            tc.tile_pool(name="consts", bufs=1) as cpool,
            tc.tile_pool(name="xt", bufs=NL) as xpool,
            tc.tile_pool(name="outp", bufs=ot_bufs) as opool,
            tc.tile_pool(name="ps", bufs=ps_bufs, space="PSUM") as pspool,
        ):
            # constants: vk heads the scalar queue
            vk_sb = cpool.tile([P, JC * NS], fp8, tag="vk")
            nc.scalar.dma_start(
                out=vk_sb.rearrange("p (c s) -> p c s", c=JC),
                in_=Vk.rearrange("(c p) s -> p c s", p=P),
            )
            vk3 = vk_sb.rearrange("p (c s) -> p c s", c=JC)

            # all slice loads issued up-front, alternating queues
            xts = []
            for t in range(NL):
                xt_sb = xpool.tile([P, JC * TW], fp8, tag="xt",
                                   name=f"xt{t}")
                ld_eng = nc.sync if t % 2 == 0 else nc.scalar
                ld_eng.dma_start(out=xt_sb, in_=xP[t])
                xts.append(xt_sb.rearrange("p (c f) -> p c f", c=JC))

            # r is tiny; gpsimd's SWDGE queue is empty so it lands early
            # without delaying the x-slice streams
            r_sb = cpool.tile([P, SB], f32, tag="r")
            nc.gpsimd.dma_start(out=r_sb, in_=rb.rearrange("(c p) -> p c", p=P))

            # warmups while loads land: ramp the PE p-state with dummy
            # matmuls and pull scalar's ACT_TABLE_LOAD off the critical path
            scr = cpool.tile([P, 512], fp8, tag="scr")
            nc.gpsimd.memset(scr, 0)
            scr2 = cpool.tile([P, 512], f32, tag="scr2")
            nc.scalar.add(scr2[:, :1], scr[:, :1], 0.0)
            wps = pspool.tile([P, 512], f32, tag="ps0", name="wps")
            for w in range(7):
                nc.tensor.matmul(wps, lhsT=scr[:, :P], rhs=scr,
                                 start=True, stop=True)

            for tp in range(NL // 2):
                ots = [opool.tile([P, 2 * TW], bf16, tag=f"ot{sb}",
                                  name=f"ot{sb}")
                       for sb in range(SB)]
                for ti in range(2):
                    t = 2 * tp + ti
                    for sb in range(SB):
                        ps = pspool.tile([P, TW], f32, tag=f"ps{sb}",
                                         name=f"ps{sb}")
                        for cp in range(CP):
                            nc.tensor.matmul(
                                ps,
                                lhsT=vk3[:, 2 * cp:2 * cp + 2,
                                         sb * P:(sb + 1) * P],
                                rhs=xts[t][:, 2 * cp:2 * cp + 2, :],
                                start=(cp == 0),
                                stop=(cp == CP - 1),
                                perf_mode=mybir.MatmulPerfMode.DoubleRow,
                            )
                        osl = slice(ti * TW, (ti + 1) * TW)
                        if (t * SB + sb) % 2 == 0:
                            nc.vector.tensor_scalar_add(ots[sb][:, osl], ps,
                                                        r_sb[:, sb:sb + 1])
                        else:
                            nc.scalar.add(ots[sb][:, osl], ps,
                                          r_sb[:, sb:sb + 1])
                for sb in range(SB):
                    st_eng = nc.sync if sb == 0 else nc.gpsimd
                    st_eng.dma_start(
                        out=outU[sb * P:(sb + 1) * P,
                                 2 * tp * TW:2 * (tp + 1) * TW],
                        in_=ots[sb],
                    )
    nc.compile()
    return nc


_NC_CACHE = {}


def _get_nc():
    if "nc" not in _NC_CACHE:
        _NC_CACHE["nc"] = _build_kernel()
    return _NC_CACHE["nc"]


def _derive_operator(A, unknown, mask):
    """Compose the scan into (Vk, rS, known) in float64."""
    A64 = np.asarray(A, dtype=np.float64)
    deg = A64.sum(axis=1)
    M = np.eye(N, dtype=np.float64)
    for u in unknown:
        M[:, u] = M @ (A64[u] / deg[u])
    r = float(mask) * M[unknown, :].sum(axis=0)
    M[unknown, :] = 0.0
    known = np.setdiff1d(np.arange(N, dtype=np.int64), unknown)
    Vk = M[known][:, unknown].astype(FP8)
    rS = np.ascontiguousarray(r[unknown], dtype=np.float32)
    return Vk, rS, known


def _prep_in_maps(x, Vk, rS, known):
    in_maps = []
    for b in range(B):
        xT = np.ascontiguousarray(x[b].T[known]).astype(FP8)  # [768, 4096]
        # slice-major tiling: xP[l, p, c*TW + u] = xT[c*128 + p, l*TW + u]
        xP = np.ascontiguousarray(
            xT.reshape(JC, P, NL, TW).transpose(2, 1, 0, 3)
        ).reshape(NL, P, JC * TW)
        in_maps.append({"xP": xP, "Vk": Vk, "r": rS})
    return in_maps


def kernel(input, A, unknown, mask, _spmd_kwargs=None):
    x = np.asarray(input, dtype=np.float32)
    unknown = np.asarray(unknown).astype(np.int64)
    Vk, rS, known = _derive_operator(A, unknown, mask)
    in_maps = _prep_in_maps(x, Vk, rS, known)

    nc = _get_nc()
    res = run_bass_kernel_spmd(nc, in_maps, core_ids=list(range(B)),
                               **(_spmd_kwargs or {}))

    out = x.copy()
    for b in range(B):
        out[b][:, unknown] = res.results[b]["outU"].T.astype(np.float32)
    return out
